# revision 36
# baseline (speedup 1.0000x reference)
"""Trainium2 Bass kernel for the AdaptiveFourierNeuralOperator problem.

Math (all derived host-side, validated vs the reference):
  xc  = rfft(x, ortho)                  -> folded into layer-1 weights: W1R/W1I = D @ w1 composites
  moe rank-1 paths                      -> never materialized; u1/u2 coefficient rows flow through
                                           precomputed composite matrices into one small fused matmul
  irfft                                 -> folded into layer-2 weights: W2R/W2I = w2 @ E composites
  softmax gate (att) and null mask      -> tiny [32x4]; computed host-side and folded into the
                                           per-batch rank-small weight slabs (exact algebra)

Device layout: [feature, seq] on-chip; x is transposed once at input via PE
transpose-mode (bf16, packed 8 blocks per PSUM bank); layer-2 emits [seq, feature]
directly so stores are contiguous.  Sharding: data-parallel over batch, 4 per core.

Measured on 8 axon TRN2 cores: HW exec ~108 us, rel L2 err 3.9e-3 (gate 2e-2).
Roughly: ~11 us NEFF fixed overhead + ~12 us startup chain + PE-bound body
(~65 us of warm matmul streams + clock-warmup/contention stretch).
"""

import sys
import types

import numpy as np
import ml_dtypes

import concourse.bass as bass
from concourse import bacc
import concourse.mybir as mybir
from concourse.masks import make_identity
from concourse.bass_utils import run_bass_kernel_spmd
from concourse.tile import TileContext

B, N, C, G = 32, 2048, 256, 4
F = C // 2 + 1          # 129
LORA = 4.0
N_CORES = 8
BPC = B // N_CORES      # batches per core = 4
GRP = 1024              # rows per group (1MB fp32 DMA)
NGRP = N // GRP         # groups per batch = 2
ROWS = BPC * N          # 8192 rows per core

BF16 = mybir.dt.bfloat16
FP32 = mybir.dt.float32


# ---------------------------------------------------------------- host math
def _host_precompute(inputs):
    f64 = np.float64
    w1 = inputs["w1"].astype(f64)
    b1 = inputs["b1"].astype(f64)
    w2 = inputs["w2"].astype(f64)
    b2 = inputs["b2"].astype(f64)
    emb_w = inputs["emb_w"].astype(f64)
    emb_b = inputs["emb_b"].astype(f64)
    gf = inputs["gra_feature"].astype(f64)
    A1 = inputs["A1r"].astype(f64) + 1j * inputs["A1i"].astype(f64)
    B1 = inputs["B1r"].astype(f64) + 1j * inputs["B1i"].astype(f64)
    A2 = inputs["A2r"].astype(f64) + 1j * inputs["A2i"].astype(f64)
    B2 = inputs["B2r"].astype(f64) + 1j * inputs["B2i"].astype(f64)
    tg = inputs["time_gra"].astype(f64)

    cc = np.arange(C)[:, None].astype(f64)
    ff = np.arange(F)[None, :].astype(f64)
    ang = -2.0 * np.pi * cc * ff / C
    Dr = np.cos(ang) / np.sqrt(C)
    Di = np.sin(ang) / np.sqrt(C)
    wgt = np.full(F, 2.0); wgt[0] = 1.0; wgt[-1] = 1.0
    tt = np.arange(C)[None, :].astype(f64)
    ang2 = 2.0 * np.pi * ff.T * tt / C
    Er = (wgt[:, None] * np.cos(ang2)) / np.sqrt(C)
    Ei = (-wgt[:, None] * np.sin(ang2)) / np.sqrt(C)

    W1R = Dr @ w1[0] - Di @ w1[1]            # [C, F]
    W1I = Dr @ w1[1] + Di @ w1[0]
    W2R = w2[0] @ Er + w2[1] @ Ei            # [F, C]
    W2I = w2[0] @ Ei - w2[1] @ Er
    bias_row = b2[0] @ Er + b2[1] @ Ei       # [C]

    Dc = Dr + 1j * Di
    d1 = Dc @ B1.T                           # [C, G]
    P = A1 @ B2.T                            # [G, G]

    u1r_y = A1.real @ W2R + A1.imag @ W2I    # [G, C]
    u1i_y = -A1.imag @ W2R + A1.real @ W2I
    u2r_y = A2.real @ Er + A2.imag @ Ei
    u2i_y = -A2.imag @ Er + A2.real @ Ei

    gra = tg @ emb_w + emb_b
    logits = gra @ gf.T
    e = np.exp(logits - logits.max(axis=1, keepdims=True))
    att = (e / e.sum(axis=1, keepdims=True)).T         # [G, B]
    att = att * (tg.sum(axis=1) != 0)[None, :]         # null mask folds in

    bf = ml_dtypes.bfloat16

    # shared constants
    # [128, 2*128]: free = (ci, m) so chunk ci is cols 128ci..128ci+128
    w1a = np.concatenate([W1R[0:128, 0:128], W1R[128:256, 0:128]], axis=1).astype(bf)
    w1b = np.concatenate([W1I[0:128, 0:128], W1I[128:256, 0:128]], axis=1).astype(bf)
    w2r = W2R[0:128, :].astype(bf)                                        # [128,256]
    w2i = W2I[0:128, :].astype(bf)
    # rows 0,1: s1 nyquist; 2-9: u1s; 32-39: u2s; 63: bias via ones row.
    # Padding rows are zero (stk padding rows are memset to 0 on device).
    wsmall = np.zeros((64, C), dtype=f64)
    wsmall[0] = W2R[128]; wsmall[1] = W2I[128]
    wsmall[2:6] = u1r_y; wsmall[6:10] = u1i_y
    wsmall[32:36] = u2r_y; wsmall[36:40] = u2i_y
    wsmall[63] = bias_row
    wsmall = wsmall.astype(bf)
    b1a = b1[0][0:128, None].astype(np.float32)  # [128,1]
    b1b = b1[1][0:128, None].astype(np.float32)
    bstk = np.zeros((10, 1), dtype=np.float32)
    bstk[0, 0] = b1[0][128]
    bstk[1, 0] = b1[1][128]
    fstk = np.full((10, 1), -3.0e38, dtype=np.float32)
    fstk[0:2, 0] = 0.0

    # per-batch slabs (att * LORA folded; null batches come out zero)
    w1c = np.zeros((B, 2, 128, 10), dtype=f64)
    u2s0 = np.zeros((B, 128, 8), dtype=f64)
    u2s1 = np.zeros((B, 128, 8), dtype=f64)
    u2s2 = np.zeros((B, 10, 8), dtype=f64)
    for b in range(B):
        sc = att[:, b] * LORA                     # [G]
        for ci in range(2):
            sl = slice(128 * ci, 128 * (ci + 1))
            w1c[b, ci, :, 0] = W1R[sl, 128]
            w1c[b, ci, :, 1] = W1I[sl, 128]
            w1c[b, ci, :, 2:6] = sc * d1.real[sl]
            w1c[b, ci, :, 6:10] = sc * d1.imag[sl]
        u2s0[b, :, 0:4] = sc * B2.real.T[0:128]   # rhs=s1r -> u2r
        u2s0[b, :, 4:8] = sc * B2.imag.T[0:128]   # rhs=s1r -> u2i
        u2s1[b, :, 0:4] = -sc * B2.imag.T[0:128]  # rhs=s1i -> u2r
        u2s1[b, :, 4:8] = sc * B2.real.T[0:128]   # rhs=s1i -> u2i
        # stacked rows 0,1 = s1 nyquist r/i; rows 2-9 = u1s
        u2s2[b, 0, 0:4] = sc * B2.real[:, 128]
        u2s2[b, 0, 4:8] = sc * B2.imag[:, 128]
        u2s2[b, 1, 0:4] = -sc * B2.imag[:, 128]
        u2s2[b, 1, 4:8] = sc * B2.real[:, 128]
        u2s2[b, 2:6, 0:4] = sc * P.real
        u2s2[b, 2:6, 4:8] = sc * P.imag
        u2s2[b, 6:10, 0:4] = -sc * P.imag
        u2s2[b, 6:10, 4:8] = sc * P.real

    shared = dict(w1a=w1a, w1b=w1b, w2r=w2r, w2i=w2i, wsmall=wsmall,
                  b1a=b1a, b1b=b1b, bstk=bstk, fstk=fstk,
                  stkinit=_stkinit(bf))
    per_batch = dict(w1c=w1c.astype(bf), u2s0=u2s0.astype(bf),
                     u2s1=u2s1.astype(bf), u2s2=u2s2.astype(bf))
    return shared, per_batch


def _stkinit(bf):
    a = np.zeros((64, GRP), dtype=bf)
    a[63] = 1.0
    return a


def _core_layout(per_batch, i):
    """Slice per-batch slabs for core i and flatten to the 2D layouts the
    graph expects: w1c [128, 2*BPC*10] free=(ci,b,m); u2s* [*, BPC*8]."""
    s = slice(BPC * i, BPC * (i + 1))
    w1c = per_batch["w1c"][s]                       # [BPC, 2, 128, 10]
    w1c2 = np.ascontiguousarray(
        w1c.transpose(2, 1, 0, 3).reshape(128, 2 * BPC * 10))
    u2s0 = np.ascontiguousarray(
        per_batch["u2s0"][s].transpose(1, 0, 2).reshape(128, BPC * 8))
    u2s1 = np.ascontiguousarray(
        per_batch["u2s1"][s].transpose(1, 0, 2).reshape(128, BPC * 8))
    u2s2 = np.ascontiguousarray(
        per_batch["u2s2"][s].transpose(1, 0, 2).reshape(10, BPC * 8))
    return dict(w1c=w1c2, u2s0=u2s0, u2s1=u2s1, u2s2=u2s2)


# ---------------------------------------------------------------- device graph
_NC_CACHE = {}


def _build():
    if "nc" in _NC_CACHE:
        return _NC_CACHE["nc"]
    nc = bacc.Bacc(None, target_bir_lowering=False)

    x = nc.dram_tensor("x", [ROWS, C], BF16, kind="ExternalInput")
    out = nc.dram_tensor("out", [ROWS, C], FP32, kind="ExternalOutput")
    d_w1a = nc.dram_tensor("w1a", [128, 256], BF16, kind="ExternalInput")
    d_w1b = nc.dram_tensor("w1b", [128, 256], BF16, kind="ExternalInput")
    d_w1c = nc.dram_tensor("w1c", [128, 2 * BPC * 10], BF16, kind="ExternalInput")
    d_w2r = nc.dram_tensor("w2r", [128, C], BF16, kind="ExternalInput")
    d_w2i = nc.dram_tensor("w2i", [128, C], BF16, kind="ExternalInput")
    d_wsm = nc.dram_tensor("wsmall", [64, C], BF16, kind="ExternalInput")
    d_b1a = nc.dram_tensor("b1a", [128, 1], FP32, kind="ExternalInput")
    d_b1b = nc.dram_tensor("b1b", [128, 1], FP32, kind="ExternalInput")
    d_bstk = nc.dram_tensor("bstk", [10, 1], FP32, kind="ExternalInput")
    d_fstk = nc.dram_tensor("fstk", [10, 1], FP32, kind="ExternalInput")
    d_u2s0 = nc.dram_tensor("u2s0", [128, BPC * 8], BF16, kind="ExternalInput")
    d_u2s1 = nc.dram_tensor("u2s1", [128, BPC * 8], BF16, kind="ExternalInput")
    d_u2s2 = nc.dram_tensor("u2s2", [10, BPC * 8], BF16, kind="ExternalInput")
    d_stkinit = nc.dram_tensor("stkinit", [64, GRP], BF16, kind="ExternalInput")

    RELU = mybir.ActivationFunctionType.Relu
    COPY = mybir.ActivationFunctionType.Copy

    with TileContext(nc) as tc:
        with (
            tc.tile_pool(name="const", bufs=1) as cpool,
            tc.tile_pool(name="io", bufs=4) as iopool,
            tc.tile_pool(name="work", bufs=3) as wpool,
            tc.tile_pool(name="pst", bufs=2, space="PSUM") as pst,
            tc.tile_pool(name="psab", bufs=2, space="PSUM") as psab,
            tc.tile_pool(name="psc", bufs=1, space="PSUM") as pscp,
            tc.tile_pool(name="psu", bufs=1, space="PSUM") as psup,
            tc.tile_pool(name="psy", bufs=2, space="PSUM") as psyp,
        ):
            # ---- constants into SBUF
            # HAM warmup: dep-free dummy matmuls so the PE clock-gate opens
            # before the first real matmul (saves ~10us of half-clock time).
            wut = cpool.tile([128, 128], BF16, tag="wut")
            nc.gpsimd.memset(wut[:, :], 1.0)
            wup = psyp.tile([128, 128], FP32, tag="y")
            for _ in range(40):
                nc.tensor.matmul(wup[:, :], wut[:, :], wut[:, :],
                                 start=True, stop=True)
            ident = cpool.tile([128, 128], BF16, tag="ident")
            make_identity(nc, ident[:, :])
            t_w1a = cpool.tile([128, 256], BF16, tag="w1a")
            nc.sync.dma_start(out=t_w1a[:, :], in_=d_w1a[:, :])
            t_w1b = cpool.tile([128, 256], BF16, tag="w1b")
            nc.gpsimd.dma_start(out=t_w1b[:, :], in_=d_w1b[:, :])
            t_w1c = cpool.tile([128, 2 * BPC * 10], BF16, tag="w1c")
            nc.scalar.dma_start(out=t_w1c[:, :], in_=d_w1c[:, :])
            t_w2r = cpool.tile([128, C], BF16, tag="w2r")
            nc.sync.dma_start(out=t_w2r[:, :], in_=d_w2r[:, :])
            t_w2i = cpool.tile([128, C], BF16, tag="w2i")
            nc.gpsimd.dma_start(out=t_w2i[:, :], in_=d_w2i[:, :])
            t_wsm = cpool.tile([64, C], BF16, tag="wsm")
            nc.scalar.dma_start(out=t_wsm[:, :], in_=d_wsm[:, :])
            t_b1a = cpool.tile([128, 1], FP32, tag="b1a")
            nc.gpsimd.dma_start(out=t_b1a[:, :], in_=d_b1a[:, :])
            t_b1b = cpool.tile([128, 1], FP32, tag="b1b")
            nc.sync.dma_start(out=t_b1b[:, :], in_=d_b1b[:, :])
            t_bstk = cpool.tile([10, 1], FP32, tag="bstk")
            nc.scalar.dma_start(out=t_bstk[:, :], in_=d_bstk[:, :])
            t_fstk = cpool.tile([10, 1], FP32, tag="fstk")
            nc.scalar.dma_start(out=t_fstk[:, :], in_=d_fstk[:, :])
            t_u2s0 = cpool.tile([128, BPC * 8], BF16, tag="u2s0")
            nc.sync.dma_start(out=t_u2s0[:, :], in_=d_u2s0[:, :])
            t_u2s1 = cpool.tile([128, BPC * 8], BF16, tag="u2s1")
            nc.gpsimd.dma_start(out=t_u2s1[:, :], in_=d_u2s1[:, :])
            t_u2s2 = cpool.tile([10, BPC * 8], BF16, tag="u2s2")
            nc.sync.dma_start(out=t_u2s2[:, :], in_=d_u2s2[:, :])
            # persistent stacked tiles (ping-pong): padding rows stay zero,
            # row 63 stays ones; per-group writers only touch rows 0-9, 32-39
            stks = []
            for si in range(2):
                st = cpool.tile([64, GRP], BF16, tag=f"stk{si}")
                nc.scalar.dma_start(out=st[:, :], in_=d_stkinit[:, :])
                stks.append(st)

            # ---- per-group pipeline (group = 1024 rows)
            for b in range(BPC):
                for h in range(NGRP):
                    gi = b * NGRP + h
                    base = b * N + h * GRP
                    src = x[base:base + GRP, :].rearrange(
                        "(p e) c -> p (e c)", p=128)
                    xg = iopool.tile([128, 2 * GRP], BF16, tag="xg")
                    ldeng = nc.sync
                    ldeng.dma_start(out=xg[:, :], in_=src)

                    xt0 = wpool.tile([128, GRP], BF16, tag="xt0")
                    xt1 = wpool.tile([128, GRP], BF16, tag="xt1")
                    # 16 PE transposes, packed 8-per-bank, same c-half
                    for half in range(2):
                        pt = pst.tile([128, GRP], BF16, tag="pt")
                        for j in range(8):
                            m = 2 * j + half
                            nc.tensor.transpose(
                                pt[:, 128 * j:128 * (j + 1)],
                                xg[:, 128 * m:128 * (m + 1)],
                                ident[:, :],
                            )
                        dst = xt0 if half == 0 else xt1
                        if half == 0:
                            nc.scalar.activation(dst[:, :], pt[:, :], COPY)
                        else:
                            nc.vector.tensor_copy(dst[:, :], pt[:, :])

                    s1r = wpool.tile([128, GRP], BF16, tag="s1r")
                    s1i = wpool.tile([128, GRP], BF16, tag="s1i")
                    stk = stks[gi % 2]

                    for q in range(2):
                        sl = slice(512 * q, 512 * (q + 1))
                        psa = psab.tile([128, 512], FP32, tag="ab")
                        nc.tensor.matmul(psa[:, :], t_w1a[:, 0:128], xt0[:, sl],
                                         start=True, stop=False)
                        nc.tensor.matmul(psa[:, :], t_w1a[:, 128:256], xt1[:, sl],
                                         start=False, stop=True)
                        nc.scalar.activation(s1r[:, sl], psa[:, :], RELU,
                                             bias=t_b1a[:, 0:1])
                        psb = psab.tile([128, 512], FP32, tag="ab")
                        nc.tensor.matmul(psb[:, :], t_w1b[:, 0:128], xt0[:, sl],
                                         start=True, stop=False)
                        nc.tensor.matmul(psb[:, :], t_w1b[:, 128:256], xt1[:, sl],
                                         start=False, stop=True)
                        nc.scalar.activation(s1i[:, sl], psb[:, :], RELU,
                                             bias=t_b1b[:, 0:1])
                        psc = pscp.tile([10, 512], FP32, tag="c")
                        nc.tensor.matmul(psc[:, :],
                                         t_w1c[:, 10 * b:10 * (b + 1)],
                                         xt0[:, sl], start=True, stop=False)
                        nc.tensor.matmul(psc[:, :],
                                         t_w1c[:, BPC * 10 + 10 * b:BPC * 10 + 10 * (b + 1)],
                                         xt1[:, sl], start=False, stop=True)
                        nc.vector.tensor_scalar(
                            stk[0:10, sl], psc[0:10, :],
                            t_bstk[:, 0:1], t_fstk[:, 0:1],
                            op0=mybir.AluOpType.add,
                            op1=mybir.AluOpType.max)

                        psu = psup.tile([8, 512], FP32, tag="u")
                        nc.tensor.matmul(psu[:, :],
                                         t_u2s0[:, 8 * b:8 * (b + 1)],
                                         s1r[:, sl], start=True, stop=False)
                        nc.tensor.matmul(psu[:, :],
                                         t_u2s1[:, 8 * b:8 * (b + 1)],
                                         s1i[:, sl], start=False, stop=False)
                        nc.tensor.matmul(psu[:, :],
                                         t_u2s2[:, 8 * b:8 * (b + 1)],
                                         stk[0:10, sl], start=False, stop=True)
                        nc.vector.tensor_copy(stk[32:40, sl], psu[:, :])

                    og = iopool.tile([128, 2 * GRP], FP32, tag="og")
                    for tp in range(4):   # two n-tiles per psum bank
                        psy = psyp.tile([128, 512], FP32, tag="y")
                        for u in range(2):
                            t = 2 * tp + u
                            tsl = slice(128 * t, 128 * (t + 1))
                            halfp = psy[:, 256 * u:256 * (u + 1)]
                            nc.tensor.matmul(halfp, s1r[:, tsl], t_w2r[:, :],
                                             start=True, stop=False)
                            nc.tensor.matmul(halfp, s1i[:, tsl], t_w2i[:, :],
                                             start=False, stop=False)
                            nc.tensor.matmul(halfp, stk[:, tsl], t_wsm[:, :],
                                             start=False, stop=True)
                        if tp % 2 == 0:
                            nc.vector.tensor_copy(
                                og[:, 512 * tp:512 * (tp + 1)], psy[:, :])
                        else:
                            nc.scalar.activation(
                                og[:, 512 * tp:512 * (tp + 1)], psy[:, :], COPY)

                    dstv = out[base:base + GRP, :].rearrange(
                        "(p e) c -> p e c", p=128)
                    for z in range(2):
                        steng = nc.scalar if z == 0 else nc.sync
                        steng.dma_start(
                            out=dstv[:, 4 * z:4 * (z + 1), :],
                            in_=og[:, 1024 * z:1024 * (z + 1)].rearrange(
                                "p (e c) -> p e c", c=C))

    nc.compile()
    _NC_CACHE["nc"] = nc
    return nc


# ---------------------------------------------------------------- entry points
def _make_in_maps(inputs):
    shared, per_batch = _host_precompute(inputs)
    x = np.asarray(inputs["x"], dtype=np.float32).astype(ml_dtypes.bfloat16)
    in_maps = []
    for i in range(N_CORES):
        m = dict(shared)
        m["x"] = x[BPC * i:BPC * (i + 1)].reshape(ROWS, C)
        m.update(_core_layout(per_batch, i))
        in_maps.append(m)
    return in_maps


def kernel(**inputs):
    nc = _build()
    in_maps = _make_in_maps(inputs)
    res = run_bass_kernel_spmd(nc, in_maps, core_ids=list(range(N_CORES)))
    out = np.concatenate(
        [r["out"].reshape(BPC, N, C) for r in res.results], axis=0)
    return out.astype(np.float32)


def run_traced(inputs):
    """For test.py: run with NTFF profiling, return (out, exec_time_ns)."""
    _install_ntff_hook()
    import concourse.bass_utils as bass_utils
    bass_utils.upload_artifacts = lambda tmpdir: f"local:{tmpdir}"
    nc = _build()
    in_maps = _make_in_maps(inputs)
    res = run_bass_kernel_spmd(nc, in_maps, core_ids=list(range(N_CORES)),
                               trace=True)
    out = np.concatenate(
        [r["out"].reshape(BPC, N, C) for r in res.results], axis=0)
    return out.astype(np.float32), res.exec_time_ns


def _install_ntff_hook():
    import antenv
    if "antenv.axon_hooks" in sys.modules:
        return
    mod = types.ModuleType("antenv.axon_hooks")
    state = {"hook": None}
    mod.set_axon_ntff_profile_hook = lambda h: state.__setitem__("hook", h)
    mod.get_axon_ntff_profile_hook = lambda: state["hook"]
    sys.modules["antenv.axon_hooks"] = mod
    antenv.axon_hooks = mod
    from trn_agent_boot.trn_boot import _ntff_profile_via_ctypes
    mod.set_axon_ntff_profile_hook(
        _ntff_profile_via_ctypes("/opt/axon/libaxon_pjrt.so"))


# revision 37
# speedup vs baseline: 1.0065x; 1.0065x over previous
"""Trainium2 Bass kernel for the AdaptiveFourierNeuralOperator problem.

Math (all derived host-side, validated vs the reference):
  xc  = rfft(x, ortho)                  -> folded into layer-1 weights: W1R/W1I = D @ w1 composites
  moe rank-1 paths                      -> never materialized; u1/u2 coefficient rows flow through
                                           precomputed composite matrices into one small fused matmul
  irfft                                 -> folded into layer-2 weights: W2R/W2I = w2 @ E composites
  softmax gate (att) and null mask      -> tiny [32x4]; computed host-side and folded into the
                                           per-batch rank-small weight slabs (exact algebra)

Device layout: [feature, seq] on-chip; x is transposed once at input via PE
transpose-mode (bf16, packed 8 blocks per PSUM bank); layer-2 emits [seq, feature]
directly so stores are contiguous.  Sharding: data-parallel over batch, 4 per core.

Measured on 8 axon TRN2 cores: HW exec ~108 us, rel L2 err 3.9e-3 (gate 2e-2).
Roughly: ~11 us NEFF fixed overhead + ~12 us startup chain + PE-bound body
(~65 us of warm matmul streams + clock-warmup/contention stretch).
"""

import sys
import types

import numpy as np
import ml_dtypes

import concourse.bass as bass
from concourse import bacc
import concourse.mybir as mybir
from concourse.masks import make_identity
from concourse.bass_utils import run_bass_kernel_spmd
from concourse.tile import TileContext

B, N, C, G = 32, 2048, 256, 4
F = C // 2 + 1          # 129
LORA = 4.0
N_CORES = 8
BPC = B // N_CORES      # batches per core = 4
GRP = 1024              # rows per group (1MB fp32 DMA)
NGRP = N // GRP         # groups per batch = 2
ROWS = BPC * N          # 8192 rows per core

BF16 = mybir.dt.bfloat16
FP32 = mybir.dt.float32


# ---------------------------------------------------------------- host math
def _host_precompute(inputs):
    f64 = np.float64
    w1 = inputs["w1"].astype(f64)
    b1 = inputs["b1"].astype(f64)
    w2 = inputs["w2"].astype(f64)
    b2 = inputs["b2"].astype(f64)
    emb_w = inputs["emb_w"].astype(f64)
    emb_b = inputs["emb_b"].astype(f64)
    gf = inputs["gra_feature"].astype(f64)
    A1 = inputs["A1r"].astype(f64) + 1j * inputs["A1i"].astype(f64)
    B1 = inputs["B1r"].astype(f64) + 1j * inputs["B1i"].astype(f64)
    A2 = inputs["A2r"].astype(f64) + 1j * inputs["A2i"].astype(f64)
    B2 = inputs["B2r"].astype(f64) + 1j * inputs["B2i"].astype(f64)
    tg = inputs["time_gra"].astype(f64)

    cc = np.arange(C)[:, None].astype(f64)
    ff = np.arange(F)[None, :].astype(f64)
    ang = -2.0 * np.pi * cc * ff / C
    Dr = np.cos(ang) / np.sqrt(C)
    Di = np.sin(ang) / np.sqrt(C)
    wgt = np.full(F, 2.0); wgt[0] = 1.0; wgt[-1] = 1.0
    tt = np.arange(C)[None, :].astype(f64)
    ang2 = 2.0 * np.pi * ff.T * tt / C
    Er = (wgt[:, None] * np.cos(ang2)) / np.sqrt(C)
    Ei = (-wgt[:, None] * np.sin(ang2)) / np.sqrt(C)

    W1R = Dr @ w1[0] - Di @ w1[1]            # [C, F]
    W1I = Dr @ w1[1] + Di @ w1[0]
    W2R = w2[0] @ Er + w2[1] @ Ei            # [F, C]
    W2I = w2[0] @ Ei - w2[1] @ Er
    bias_row = b2[0] @ Er + b2[1] @ Ei       # [C]

    Dc = Dr + 1j * Di
    d1 = Dc @ B1.T                           # [C, G]
    P = A1 @ B2.T                            # [G, G]

    u1r_y = A1.real @ W2R + A1.imag @ W2I    # [G, C]
    u1i_y = -A1.imag @ W2R + A1.real @ W2I
    u2r_y = A2.real @ Er + A2.imag @ Ei
    u2i_y = -A2.imag @ Er + A2.real @ Ei

    gra = tg @ emb_w + emb_b
    logits = gra @ gf.T
    e = np.exp(logits - logits.max(axis=1, keepdims=True))
    att = (e / e.sum(axis=1, keepdims=True)).T         # [G, B]
    att = att * (tg.sum(axis=1) != 0)[None, :]         # null mask folds in

    bf = ml_dtypes.bfloat16

    # shared constants
    # [128, 2*128]: free = (ci, m) so chunk ci is cols 128ci..128ci+128
    w1a = np.concatenate([W1R[0:128, 0:128], W1R[128:256, 0:128]], axis=1).astype(bf)
    w1b = np.concatenate([W1I[0:128, 0:128], W1I[128:256, 0:128]], axis=1).astype(bf)
    w2r = W2R[0:128, :].astype(bf)                                        # [128,256]
    w2i = W2I[0:128, :].astype(bf)
    # rows 0,1: s1 nyquist; 2-9: u1s; 32-39: u2s; 63: bias via ones row.
    # Padding rows are zero (stk padding rows are memset to 0 on device).
    wsmall = np.zeros((64, C), dtype=f64)
    wsmall[0] = W2R[128]; wsmall[1] = W2I[128]
    wsmall[2:6] = u1r_y; wsmall[6:10] = u1i_y
    wsmall[32:36] = u2r_y; wsmall[36:40] = u2i_y
    wsmall[63] = bias_row
    wsmall = wsmall.astype(bf)
    b1a = b1[0][0:128, None].astype(np.float32)  # [128,1]
    b1b = b1[1][0:128, None].astype(np.float32)
    bstk = np.zeros((10, 1), dtype=np.float32)
    bstk[0, 0] = b1[0][128]
    bstk[1, 0] = b1[1][128]
    fstk = np.full((10, 1), -3.0e38, dtype=np.float32)
    fstk[0:2, 0] = 0.0

    # per-batch slabs (att * LORA folded; null batches come out zero)
    w1c = np.zeros((B, 2, 128, 10), dtype=f64)
    u2s0 = np.zeros((B, 128, 8), dtype=f64)
    u2s1 = np.zeros((B, 128, 8), dtype=f64)
    u2s2 = np.zeros((B, 10, 8), dtype=f64)
    for b in range(B):
        sc = att[:, b] * LORA                     # [G]
        for ci in range(2):
            sl = slice(128 * ci, 128 * (ci + 1))
            w1c[b, ci, :, 0] = W1R[sl, 128]
            w1c[b, ci, :, 1] = W1I[sl, 128]
            w1c[b, ci, :, 2:6] = sc * d1.real[sl]
            w1c[b, ci, :, 6:10] = sc * d1.imag[sl]
        u2s0[b, :, 0:4] = sc * B2.real.T[0:128]   # rhs=s1r -> u2r
        u2s0[b, :, 4:8] = sc * B2.imag.T[0:128]   # rhs=s1r -> u2i
        u2s1[b, :, 0:4] = -sc * B2.imag.T[0:128]  # rhs=s1i -> u2r
        u2s1[b, :, 4:8] = sc * B2.real.T[0:128]   # rhs=s1i -> u2i
        # stacked rows 0,1 = s1 nyquist r/i; rows 2-9 = u1s
        u2s2[b, 0, 0:4] = sc * B2.real[:, 128]
        u2s2[b, 0, 4:8] = sc * B2.imag[:, 128]
        u2s2[b, 1, 0:4] = -sc * B2.imag[:, 128]
        u2s2[b, 1, 4:8] = sc * B2.real[:, 128]
        u2s2[b, 2:6, 0:4] = sc * P.real
        u2s2[b, 2:6, 4:8] = sc * P.imag
        u2s2[b, 6:10, 0:4] = -sc * P.imag
        u2s2[b, 6:10, 4:8] = sc * P.real

    shared = dict(w1a=w1a, w1b=w1b, w2r=w2r, w2i=w2i, wsmall=wsmall,
                  b1a=b1a, b1b=b1b, bstk=bstk, fstk=fstk,
                  stkinit=_stkinit(bf))
    per_batch = dict(w1c=w1c.astype(bf), u2s0=u2s0.astype(bf),
                     u2s1=u2s1.astype(bf), u2s2=u2s2.astype(bf))
    return shared, per_batch


def _stkinit(bf):
    a = np.zeros((64, GRP), dtype=bf)
    a[63] = 1.0
    return a


def _core_layout(per_batch, i):
    """Slice per-batch slabs for core i and flatten to the 2D layouts the
    graph expects: w1c [128, 2*BPC*10] free=(ci,b,m); u2s* [*, BPC*8]."""
    s = slice(BPC * i, BPC * (i + 1))
    w1c = per_batch["w1c"][s]                       # [BPC, 2, 128, 10]
    w1c2 = np.ascontiguousarray(
        w1c.transpose(2, 1, 0, 3).reshape(128, 2 * BPC * 10))
    u2s0 = np.ascontiguousarray(
        per_batch["u2s0"][s].transpose(1, 0, 2).reshape(128, BPC * 8))
    u2s1 = np.ascontiguousarray(
        per_batch["u2s1"][s].transpose(1, 0, 2).reshape(128, BPC * 8))
    u2s2 = np.ascontiguousarray(
        per_batch["u2s2"][s].transpose(1, 0, 2).reshape(10, BPC * 8))
    return dict(w1c=w1c2, u2s0=u2s0, u2s1=u2s1, u2s2=u2s2)


# ---------------------------------------------------------------- device graph
_NC_CACHE = {}


def _build():
    if "nc" in _NC_CACHE:
        return _NC_CACHE["nc"]
    nc = bacc.Bacc(None, target_bir_lowering=False)

    x = nc.dram_tensor("x", [ROWS, C], BF16, kind="ExternalInput")
    out = nc.dram_tensor("out", [ROWS, C], FP32, kind="ExternalOutput")
    d_w1a = nc.dram_tensor("w1a", [128, 256], BF16, kind="ExternalInput")
    d_w1b = nc.dram_tensor("w1b", [128, 256], BF16, kind="ExternalInput")
    d_w1c = nc.dram_tensor("w1c", [128, 2 * BPC * 10], BF16, kind="ExternalInput")
    d_w2r = nc.dram_tensor("w2r", [128, C], BF16, kind="ExternalInput")
    d_w2i = nc.dram_tensor("w2i", [128, C], BF16, kind="ExternalInput")
    d_wsm = nc.dram_tensor("wsmall", [64, C], BF16, kind="ExternalInput")
    d_b1a = nc.dram_tensor("b1a", [128, 1], FP32, kind="ExternalInput")
    d_b1b = nc.dram_tensor("b1b", [128, 1], FP32, kind="ExternalInput")
    d_bstk = nc.dram_tensor("bstk", [10, 1], FP32, kind="ExternalInput")
    d_fstk = nc.dram_tensor("fstk", [10, 1], FP32, kind="ExternalInput")
    d_u2s0 = nc.dram_tensor("u2s0", [128, BPC * 8], BF16, kind="ExternalInput")
    d_u2s1 = nc.dram_tensor("u2s1", [128, BPC * 8], BF16, kind="ExternalInput")
    d_u2s2 = nc.dram_tensor("u2s2", [10, BPC * 8], BF16, kind="ExternalInput")
    d_stkinit = nc.dram_tensor("stkinit", [64, GRP], BF16, kind="ExternalInput")

    RELU = mybir.ActivationFunctionType.Relu
    COPY = mybir.ActivationFunctionType.Copy

    with TileContext(nc) as tc:
        with (
            tc.tile_pool(name="const", bufs=1) as cpool,
            tc.tile_pool(name="io", bufs=4) as iopool,
            tc.tile_pool(name="work", bufs=3) as wpool,
            tc.tile_pool(name="pst", bufs=2, space="PSUM") as pst,
            tc.tile_pool(name="psab", bufs=2, space="PSUM") as psab,
            tc.tile_pool(name="psc", bufs=1, space="PSUM") as pscp,
            tc.tile_pool(name="psu", bufs=1, space="PSUM") as psup,
            tc.tile_pool(name="psy", bufs=2, space="PSUM") as psyp,
        ):
            # ---- constants into SBUF
            # HAM warmup: dep-free dummy matmuls so the PE clock-gate opens
            # before the first real matmul (saves ~10us of half-clock time).
            wut = cpool.tile([128, 128], BF16, tag="wut")
            nc.gpsimd.memset(wut[:, :], 1.0)
            wup = psyp.tile([128, 128], FP32, tag="y")
            for _ in range(40):
                nc.tensor.matmul(wup[:, :], wut[:, :], wut[:, :],
                                 start=True, stop=True)
            ident = cpool.tile([128, 128], BF16, tag="ident")
            make_identity(nc, ident[:, :])
            t_w1a = cpool.tile([128, 256], BF16, tag="w1a")
            nc.sync.dma_start(out=t_w1a[:, :], in_=d_w1a[:, :])
            t_w1b = cpool.tile([128, 256], BF16, tag="w1b")
            nc.gpsimd.dma_start(out=t_w1b[:, :], in_=d_w1b[:, :])
            t_w1c = cpool.tile([128, 2 * BPC * 10], BF16, tag="w1c")
            nc.scalar.dma_start(out=t_w1c[:, :], in_=d_w1c[:, :])
            t_w2r = cpool.tile([128, C], BF16, tag="w2r")
            nc.sync.dma_start(out=t_w2r[:, :], in_=d_w2r[:, :])
            t_w2i = cpool.tile([128, C], BF16, tag="w2i")
            nc.gpsimd.dma_start(out=t_w2i[:, :], in_=d_w2i[:, :])
            t_wsm = cpool.tile([64, C], BF16, tag="wsm")
            nc.scalar.dma_start(out=t_wsm[:, :], in_=d_wsm[:, :])
            t_b1a = cpool.tile([128, 1], FP32, tag="b1a")
            nc.gpsimd.dma_start(out=t_b1a[:, :], in_=d_b1a[:, :])
            t_b1b = cpool.tile([128, 1], FP32, tag="b1b")
            nc.sync.dma_start(out=t_b1b[:, :], in_=d_b1b[:, :])
            t_bstk = cpool.tile([10, 1], FP32, tag="bstk")
            nc.scalar.dma_start(out=t_bstk[:, :], in_=d_bstk[:, :])
            t_fstk = cpool.tile([10, 1], FP32, tag="fstk")
            nc.scalar.dma_start(out=t_fstk[:, :], in_=d_fstk[:, :])
            t_u2s0 = cpool.tile([128, BPC * 8], BF16, tag="u2s0")
            nc.sync.dma_start(out=t_u2s0[:, :], in_=d_u2s0[:, :])
            t_u2s1 = cpool.tile([128, BPC * 8], BF16, tag="u2s1")
            nc.gpsimd.dma_start(out=t_u2s1[:, :], in_=d_u2s1[:, :])
            t_u2s2 = cpool.tile([10, BPC * 8], BF16, tag="u2s2")
            nc.sync.dma_start(out=t_u2s2[:, :], in_=d_u2s2[:, :])
            # persistent stacked tiles (ping-pong): padding rows stay zero,
            # row 63 stays ones; per-group writers only touch rows 0-9, 32-39
            stks = []
            for si in range(2):
                st = cpool.tile([64, GRP], BF16, tag=f"stk{si}")
                nc.scalar.dma_start(out=st[:, :], in_=d_stkinit[:, :])
                stks.append(st)

            # ---- per-group pipeline (group = 1024 rows)
            for b in range(BPC):
                for h in range(NGRP):
                    gi = b * NGRP + h
                    base = b * N + h * GRP
                    src = x[base:base + GRP, :].rearrange(
                        "(p e) c -> p (e c)", p=128)
                    xg = iopool.tile([128, 2 * GRP], BF16, tag="xg")
                    ldeng = nc.sync
                    ldeng.dma_start(out=xg[:, :], in_=src)

                    xt0 = wpool.tile([128, GRP], BF16, tag="xt0")
                    xt1 = wpool.tile([128, GRP], BF16, tag="xt1")
                    # 16 PE transposes, packed 8-per-bank, same c-half
                    for half in range(2):
                        pt = pst.tile([128, GRP], BF16, tag="pt")
                        for j in range(8):
                            m = 2 * j + half
                            nc.tensor.transpose(
                                pt[:, 128 * j:128 * (j + 1)],
                                xg[:, 128 * m:128 * (m + 1)],
                                ident[:, :],
                            )
                        dst = xt0 if half == 0 else xt1
                        if half == 0:
                            nc.scalar.activation(dst[:, :], pt[:, :], COPY)
                        else:
                            nc.vector.tensor_copy(dst[:, :], pt[:, :])

                    s1r = wpool.tile([128, GRP], BF16, tag="s1r")
                    s1i = wpool.tile([128, GRP], BF16, tag="s1i")
                    stk = stks[gi % 2]

                    for q in range(2):
                        sl = slice(512 * q, 512 * (q + 1))
                        psa = psab.tile([128, 512], FP32, tag="ab")
                        nc.tensor.matmul(psa[:, :], t_w1a[:, 0:128], xt0[:, sl],
                                         start=True, stop=False)
                        nc.tensor.matmul(psa[:, :], t_w1a[:, 128:256], xt1[:, sl],
                                         start=False, stop=True)
                        nc.scalar.activation(s1r[:, sl], psa[:, :], RELU,
                                             bias=t_b1a[:, 0:1])
                        psb = psab.tile([128, 512], FP32, tag="ab")
                        nc.tensor.matmul(psb[:, :], t_w1b[:, 0:128], xt0[:, sl],
                                         start=True, stop=False)
                        nc.tensor.matmul(psb[:, :], t_w1b[:, 128:256], xt1[:, sl],
                                         start=False, stop=True)
                        nc.scalar.activation(s1i[:, sl], psb[:, :], RELU,
                                             bias=t_b1b[:, 0:1])
                        psc = pscp.tile([10, 512], FP32, tag="c")
                        nc.tensor.matmul(psc[:, :],
                                         t_w1c[:, 10 * b:10 * (b + 1)],
                                         xt0[:, sl], start=True, stop=False)
                        nc.tensor.matmul(psc[:, :],
                                         t_w1c[:, BPC * 10 + 10 * b:BPC * 10 + 10 * (b + 1)],
                                         xt1[:, sl], start=False, stop=True)
                        nc.vector.tensor_scalar(
                            stk[0:10, sl], psc[0:10, :],
                            t_bstk[:, 0:1], t_fstk[:, 0:1],
                            op0=mybir.AluOpType.add,
                            op1=mybir.AluOpType.max)

                        psu = psup.tile([8, 512], FP32, tag="u")
                        nc.tensor.matmul(psu[:, :],
                                         t_u2s0[:, 8 * b:8 * (b + 1)],
                                         s1r[:, sl], start=True, stop=False)
                        nc.tensor.matmul(psu[:, :],
                                         t_u2s1[:, 8 * b:8 * (b + 1)],
                                         s1i[:, sl], start=False, stop=False)
                        nc.tensor.matmul(psu[:, :],
                                         t_u2s2[:, 8 * b:8 * (b + 1)],
                                         stk[0:10, sl], start=False, stop=True)
                        nc.vector.tensor_copy(stk[32:40, sl], psu[:, :])

                    og = iopool.tile([128, 2 * GRP], FP32, tag="og")
                    for tp in range(4):   # two n-tiles per psum bank
                        psy = psyp.tile([128, 512], FP32, tag="y")
                        for u in range(2):
                            t = 2 * tp + u
                            tsl = slice(128 * t, 128 * (t + 1))
                            halfp = psy[:, 256 * u:256 * (u + 1)]
                            nc.tensor.matmul(halfp, s1r[:, tsl], t_w2r[:, :],
                                             start=True, stop=False)
                            nc.tensor.matmul(halfp, s1i[:, tsl], t_w2i[:, :],
                                             start=False, stop=False)
                            nc.tensor.matmul(halfp, stk[:, tsl], t_wsm[:, :],
                                             start=False, stop=True)
                        if tp % 2 == 0:
                            nc.vector.tensor_copy(
                                og[:, 512 * tp:512 * (tp + 1)], psy[:, :])
                        else:
                            nc.scalar.activation(
                                og[:, 512 * tp:512 * (tp + 1)], psy[:, :], COPY)

                    dstv = out[base:base + GRP, :].rearrange(
                        "(p e) c -> p e c", p=128)
                    for z in range(2):
                        nc.scalar.dma_start(
                            out=dstv[:, 4 * z:4 * (z + 1), :],
                            in_=og[:, 1024 * z:1024 * (z + 1)].rearrange(
                                "p (e c) -> p e c", c=C))

    nc.compile()
    _NC_CACHE["nc"] = nc
    return nc


# ---------------------------------------------------------------- entry points
def _make_in_maps(inputs):
    shared, per_batch = _host_precompute(inputs)
    x = np.asarray(inputs["x"], dtype=np.float32).astype(ml_dtypes.bfloat16)
    in_maps = []
    for i in range(N_CORES):
        m = dict(shared)
        m["x"] = x[BPC * i:BPC * (i + 1)].reshape(ROWS, C)
        m.update(_core_layout(per_batch, i))
        in_maps.append(m)
    return in_maps


def kernel(**inputs):
    nc = _build()
    in_maps = _make_in_maps(inputs)
    res = run_bass_kernel_spmd(nc, in_maps, core_ids=list(range(N_CORES)))
    out = np.concatenate(
        [r["out"].reshape(BPC, N, C) for r in res.results], axis=0)
    return out.astype(np.float32)


def run_traced(inputs):
    """For test.py: run with NTFF profiling, return (out, exec_time_ns)."""
    _install_ntff_hook()
    import concourse.bass_utils as bass_utils
    bass_utils.upload_artifacts = lambda tmpdir: f"local:{tmpdir}"
    nc = _build()
    in_maps = _make_in_maps(inputs)
    res = run_bass_kernel_spmd(nc, in_maps, core_ids=list(range(N_CORES)),
                               trace=True)
    out = np.concatenate(
        [r["out"].reshape(BPC, N, C) for r in res.results], axis=0)
    return out.astype(np.float32), res.exec_time_ns


def _install_ntff_hook():
    import antenv
    if "antenv.axon_hooks" in sys.modules:
        return
    mod = types.ModuleType("antenv.axon_hooks")
    state = {"hook": None}
    mod.set_axon_ntff_profile_hook = lambda h: state.__setitem__("hook", h)
    mod.get_axon_ntff_profile_hook = lambda: state["hook"]
    sys.modules["antenv.axon_hooks"] = mod
    antenv.axon_hooks = mod
    from trn_agent_boot.trn_boot import _ntff_profile_via_ctypes
    mod.set_axon_ntff_profile_hook(
        _ntff_profile_via_ctypes("/opt/axon/libaxon_pjrt.so"))
